# revision 2
# baseline (speedup 1.0000x reference)
"""Trainium2 Bass kernel for the LAS-style attention LSTM decoder (v3).

Data-parallel over batch (16 of 128 per core, 8 cores), 249 unrolled steps.

Improvements over the v1 baseline:
- lens-specialized attention: batch is sorted by length so local slot b
  holds similar lengths on all cores; energy streams ceil(len/128)*128
  columns and context runs ceil(len/128) chunk-matvecs (40 vs 64 MMs).
- diag-mask tensors laid out [p, m, b] (b innermost) so the mask muls hit
  DVE 2x mode (594ns vs 1127ns for the attention mask).
- g-gate rows host-prescaled by 2 -> one fused tanh(0.5 x) per gate block
  (4+2 ACT ops per step instead of 7); gate order f,i,g,o.
- gih (embedding-gate rows) and preds move in 4-step batched DMAs with
  b-major DRAM layout; preds stored bf16, b_out added on host.
- emission order + pool bufs let next-step gih/h1 gate matmuls prefire
  into the attention-phase PE gaps (ctx contributions emitted last).
- zero bias2 dropped (asserted); output projection copies in bf16.
"""

import numpy as np
import ml_dtypes

S, B, T = 500, 128, 250
H, K, V, VOCAB = 512, 128, 128, 1000
NC = 8
BC = B // NC          # 16 batch per core
TS = T - 1            # 249 decoder steps
SP = 512
G1, G2 = 4 * H, 4 * K
TCH = 4               # steps per DMA batch

BF16N = ml_dtypes.bfloat16

_BUILT = {}


def _build(nsteps, nch):
    import concourse.bacc as bacc
    import concourse.tile as tile
    import concourse.mybir as mybir

    F32 = mybir.dt.float32
    BF16 = mybir.dt.bfloat16
    AF = mybir.ActivationFunctionType
    ALU = mybir.AluOpType

    nc = bacc.Bacc("TRN2", target_bir_lowering=False, debug=False)

    def din(name, shape, dt=BF16):
        return nc.dram_tensor(name, shape, dt, kind="ExternalInput").ap()

    whh1T_d = din("whh1T", [128, 4, G1])
    wih1cT_d = din("wih1cT", [128, G1])
    wih2T_d = din("wih2T", [128, 4, G2])
    whh2T_d = din("whh2T", [128, G2])
    woutT_d = din("woutT", [128, 2, VOCAB])
    kk_d = din("kk", [128, BC, SP])
    vv_d = din("vv", [128, 4, BC, V + 2])
    dmT_d = din("dmT", [128, BC * BC])        # [p, m, b] diag: 1 iff m==b
    i16b_d = din("i16b", [16, 16])
    gih_d = din("gih", [BC, nsteps, G1])
    preds_d = nc.dram_tensor("preds", [BC, nsteps, VOCAB], BF16,
                             kind="ExternalOutput").ap()

    n_tch = (nsteps + TCH - 1) // TCH

    with tile.TileContext(nc) as tc:
        with (
            tc.tile_pool(name="consts", bufs=1) as cp,
            tc.tile_pool(name="state", bufs=1) as sp,
            tc.tile_pool(name="work", bufs=2) as wp,
            tc.tile_pool(name="gpool", bufs=2) as gp,
            tc.tile_pool(name="popool", bufs=2) as op,
            tc.tile_pool(name="psg1", bufs=1, space="PSUM") as pg,
            tc.tile_pool(name="psg2", bufs=1, space="PSUM") as pg2p,
            tc.tile_pool(name="pse", bufs=1, space="PSUM") as pep,
            tc.tile_pool(name="psc", bufs=1, space="PSUM") as pcp,
            tc.tile_pool(name="pso", bufs=1, space="PSUM") as pop,
            tc.tile_pool(name="pstr", bufs=2, space="PSUM") as ptr,
        ):
            # ---- constants ----
            whh1T = cp.tile([128, 4, G1], BF16)
            wih1cT = cp.tile([128, G1], BF16)
            wih2T = cp.tile([128, 4, G2], BF16)
            whh2T = cp.tile([128, G2], BF16)
            woutT = cp.tile([128, 2, VOCAB], BF16)
            kk = cp.tile([128, BC, SP], BF16)
            vv = cp.tile([128, 4, BC, V + 2], BF16)
            dmT = cp.tile([128, BC * BC], BF16)
            i16b = cp.tile([16, 16], BF16)
            negC = cp.tile([16, 1], F32)
            nc.vector.memset(negC[:], -25.0)
            for t_, d_ in [(whh1T, whh1T_d), (wih1cT, wih1cT_d),
                           (wih2T, wih2T_d), (whh2T, whh2T_d),
                           (woutT, woutT_d), (kk, kk_d), (vv, vv_d),
                           (dmT, dmT_d), (i16b, i16b_d)]:
                nc.sync.dma_start(t_[:], d_[:])

            # ---- state ----
            C1 = sp.tile([16, H], F32)
            C2 = sp.tile([16, K], F32)
            h1T = sp.tile([128, 4, 16], BF16)
            h2T = sp.tile([128, 16], BF16)
            h2d = sp.tile([128, BC * BC], BF16)   # [p, m, b]
            ctxT = sp.tile([128, 16], BF16)
            for st in (C1, C2, h1T, h2T, h2d, ctxT):
                nc.vector.memset(st[:], 0.0)

            # Delayed output projection: step t's outproj is emitted during
            # step t+1 so its PE matmuls fill the elementwise-phase PE gap
            # (keeps the PE HAM clock warm) and its DVE copies run after H1.
            po_tiles = {}
            out_state = {"prev": None}   # step index whose outproj is pending

            def emit_outproj_mms(s):
                pprs = []
                for hf in range(2):
                    ppr = pop.tile([16, 512], F32, tag="po")
                    sl = slice(hf * 500, (hf + 1) * 500)
                    nc.tensor.matmul(ppr[:, 0:500], h2T[:], woutT[:, 0, sl],
                                     start=True, stop=False)
                    nc.tensor.matmul(ppr[:, 0:500], ctxT[:], woutT[:, 1, sl],
                                     start=False, stop=True)
                    pprs.append(ppr)
                return pprs

            def emit_outproj_copies(s, pprs):
                po = po_tiles[s // TCH]
                tl = s % TCH
                for hf in range(2):
                    sl = slice(hf * 500, (hf + 1) * 500)
                    nc.vector.tensor_copy(po[:, tl, sl], pprs[hf][:, 0:500])
                tch_ = min(TCH, nsteps - (s // TCH) * TCH)
                if tl == tch_ - 1:
                    t0_ = (s // TCH) * TCH
                    nc.sync.dma_start(preds_d[:, t0_:t0_ + tch_, :],
                                      po[:, 0:tch_, :])

            for tc_i in range(n_tch):
                t0 = tc_i * TCH
                tch = min(TCH, nsteps - t0)
                gbuf = gp.tile([16, TCH, G1], BF16, tag="gbuf")
                nc.sync.dma_start(gbuf[:, 0:tch, :], gih_d[:, t0:t0 + tch, :])
                po = op.tile([16, TCH, VOCAB], BF16, tag="po")
                po_tiles[tc_i] = po

                for tl in range(tch):
                    # ---- LSTM1 gates (q blocks f,i,g,o); ctx MMs last ----
                    pg1 = []
                    for q in range(4):
                        p_ = pg.tile([16, 512], F32, tag=f"g1{q % 2}")
                        pg1.append(p_)
                        sl = slice(q * 512, (q + 1) * 512)
                        nc.tensor.matmul(p_[:], i16b[:], gbuf[:, tl, sl],
                                         start=True, stop=False)
                        for c in range(4):
                            nc.tensor.matmul(p_[:], h1T[:, c, :],
                                             whh1T[:, c, sl],
                                             start=False, stop=False)
                    # ---- LSTM2: h2T part prefires ----
                    pg2 = pg2p.tile([16, G2], F32, tag="g2")
                    nc.tensor.matmul(pg2[:], h2T[:], whh2T[:],
                                     start=True, stop=False)
                    for q in range(4):
                        sl = slice(q * 512, (q + 1) * 512)
                        nc.tensor.matmul(pg1[q][:], ctxT[:], wih1cT[:, sl],
                                         start=False, stop=True)

                    # ---- delayed outproj MMs for previous step ----
                    prev_s = out_state["prev"]
                    prev_pprs = None
                    if prev_s is not None:
                        prev_pprs = emit_outproj_mms(prev_s)

                    # ---- LSTM1 elementwise ----
                    tq = []
                    for q in range(4):
                        tq_ = wp.tile([16, 512], F32, tag=f"tq{q}")
                        tq.append(tq_)
                        nc.scalar.activation(tq_[:], pg1[q][:], AF.Tanh,
                                             scale=0.5)
                    Av = wp.tile([16, H], F32, tag="Av")
                    nc.vector.scalar_tensor_tensor(Av[:], tq[0][:], 1.0,
                                                   C1[:], ALU.add, ALU.mult)
                    Bv = wp.tile([16, H], F32, tag="Bv")
                    nc.vector.scalar_tensor_tensor(Bv[:], tq[1][:], 1.0,
                                                   tq[2][:], ALU.add, ALU.mult)
                    nc.vector.scalar_tensor_tensor(C1[:], Av[:], 0.5, Bv[:],
                                                   ALU.mult, ALU.add)
                    tc1 = wp.tile([16, H], F32, tag="tc1")
                    nc.scalar.activation(tc1[:], C1[:], AF.Tanh, scale=0.5)
                    H1 = wp.tile([16, H], BF16, tag="H1")
                    nc.vector.scalar_tensor_tensor(H1[:], tq[3][:], 1.0,
                                                   tc1[:], ALU.add, ALU.mult)

                    # ---- delayed outproj copies (+ po DMA if chunk done) --
                    if prev_pprs is not None:
                        emit_outproj_copies(prev_s, prev_pprs)

                    # ---- h1 transpose ----
                    h1tr = ptr.tile([128, 1024], BF16, tag="tr")
                    for c in range(4):
                        nc.tensor.transpose(h1tr[:, c * 16:(c + 1) * 16],
                                            H1[:, c * 128:(c + 1) * 128],
                                            i16b[:])
                    nc.vector.tensor_copy(
                        h1T[:],
                        h1tr[:, 0:64].rearrange("p (c b) -> p c b", c=4))

                    # ---- LSTM2 gates finish + elementwise ----
                    for c in range(4):
                        nc.tensor.matmul(pg2[:], h1T[:, c, :], wih2T[:, c, :],
                                         start=False, stop=(c == 3))
                    tio2 = wp.tile([16, G2], F32, tag="tio2")
                    nc.scalar.activation(tio2[:], pg2[:], AF.Tanh, scale=0.5)
                    A2 = wp.tile([16, K], F32, tag="A2")
                    nc.vector.scalar_tensor_tensor(A2[:], tio2[:, 0:K], 1.0,
                                                   C2[:], ALU.add, ALU.mult)
                    B2 = wp.tile([16, K], F32, tag="B2")
                    nc.vector.scalar_tensor_tensor(B2[:], tio2[:, K:2 * K],
                                                   1.0, tio2[:, 2 * K:3 * K],
                                                   ALU.add, ALU.mult)
                    nc.vector.scalar_tensor_tensor(C2[:], A2[:], 0.5, B2[:],
                                                   ALU.mult, ALU.add)
                    tc2 = wp.tile([16, K], F32, tag="tc2")
                    nc.scalar.activation(tc2[:], C2[:], AF.Tanh, scale=0.5)
                    H2 = wp.tile([16, K], BF16, tag="H2")
                    nc.vector.scalar_tensor_tensor(H2[:], tio2[:, 3 * K:4 * K],
                                                   1.0, tc2[:],
                                                   ALU.add, ALU.mult)

                    # ---- h2 transpose + diag mask ([p, b, m] layout) ----
                    h2tr = ptr.tile([128, 1024], BF16, tag="tr")
                    nc.tensor.transpose(h2tr[:, 0:16], H2[:], i16b[:])
                    nc.vector.tensor_copy(h2T[:], h2tr[:, 0:16])
                    nc.vector.tensor_mul(
                        h2d[:].rearrange("p (b m) -> p b m", b=16),
                        h2T[:].unsqueeze(2).broadcast_to([128, 16, 16]),
                        dmT[:].rearrange("p (b m) -> p b m", b=16))

                    # ---- energy: accumulating diag matvecs [16, 512] ----
                    # descending extents: the start MM covers the widest
                    # column range so later accumulates never hit a
                    # partially-initialized psum region
                    pe = pep.tile([16, SP], F32, tag="pe")
                    for b in reversed(range(16)):
                        nb = 128 * nch[b]
                        nc.tensor.matmul(pe[:, 0:nb],
                                         h2d[:, b * 16:(b + 1) * 16],
                                         kk[:, b, 0:nb],
                                         start=(b == 15), stop=(b == 0))
                    aexp = wp.tile([16, SP], BF16, tag="aexp")
                    nc.scalar.activation(aexp[:], pe[:], AF.Exp, bias=negC[:])

                    # ---- transpose attn, diag-mask ([p, c, m, b]) ----
                    atr = ptr.tile([128, 1024], BF16, tag="tr")
                    for c in range(4):
                        nc.tensor.transpose(atr[:, c * 16 + 0:c * 16 + 16],
                                            aexp[:, c * 128:(c + 1) * 128],
                                            i16b[:])
                    ad = wp.tile([128, 4, 16, 16], BF16, tag="ad")
                    nc.vector.tensor_mul(
                        ad[:],
                        atr[:, 0:64].rearrange("p (c b) -> p c b", c=4)
                        .unsqueeze(3).broadcast_to([128, 4, 16, 16]),
                        dmT[:].rearrange("p (b m) -> p b m", b=16)
                        .unsqueeze(1).broadcast_to([128, 4, 16, 16]))

                    # ---- context: accumulating diag matvecs [16, 130] ----
                    pc = pcp.tile([16, 512], F32, tag="pc")
                    mms = [(b, c) for b in range(16) for c in range(nch[b])]
                    for k_, (b, c) in enumerate(mms):
                        nc.tensor.matmul(pc[:, 0:V + 2], ad[:, c, b, :],
                                         vv[:, c, b, :],
                                         start=(k_ == 0),
                                         stop=(k_ == len(mms) - 1))
                    rn = wp.tile([16, 1], F32, tag="rn")
                    nc.vector.reciprocal(rn[:], pc[:, V:V + 1])
                    ctxn = wp.tile([16, V], BF16, tag="ctxn")
                    nc.vector.tensor_scalar(ctxn[:], pc[:, 0:V], rn[:], None,
                                            ALU.mult)

                    # ---- ctx transpose ----
                    ctr = ptr.tile([128, 1024], BF16, tag="tr")
                    nc.tensor.transpose(ctr[:, 0:16], ctxn[:], i16b[:])
                    nc.vector.tensor_copy(ctxT[:], ctr[:, 0:16])

                    out_state["prev"] = t0 + tl

            # final step's output projection
            last_s = out_state["prev"]
            pprs = emit_outproj_mms(last_s)
            emit_outproj_copies(last_s, pprs)

    nc.finalize()
    return nc


def _host_prep(key, values, lens, text, emb, w_ih1, w_hh1, b_ih1, b_hh1,
               w_ih2, w_hh2, b_ih2, b_hh2, w_out, b_out, nsteps):
    f32 = np.float32
    key = np.asarray(key, f32)
    values = np.asarray(values, f32)
    lens = np.asarray(lens).astype(np.int64)
    text = np.asarray(text).astype(np.int64)
    emb = np.asarray(emb, f32)
    w_ih1 = np.asarray(w_ih1, f32); w_hh1 = np.asarray(w_hh1, f32)
    b_ih1 = np.asarray(b_ih1, f32); b_hh1 = np.asarray(b_hh1, f32)
    w_ih2 = np.asarray(w_ih2, f32); w_hh2 = np.asarray(w_hh2, f32)
    b_ih2 = np.asarray(b_ih2, f32); b_hh2 = np.asarray(b_hh2, f32)
    w_out = np.asarray(w_out, f32); b_out = np.asarray(b_out, f32)

    assert np.all(b_ih2 == 0) and np.all(b_hh2 == 0), "nonzero bias2"

    # gate order f,i,g,o (torch order is i,f,g,o); g-rows prescaled by 2
    permg = np.r_[H:2 * H, 0:H, 2 * H:3 * H, 3 * H:4 * H]
    permg2 = np.r_[K:2 * K, 0:K, 2 * K:3 * K, 3 * K:4 * K]
    rs1 = np.ones((G1,), f32); rs1[2 * H:3 * H] = 2.0
    rs2 = np.ones((G2,), f32); rs2[2 * K:3 * K] = 2.0

    tab1 = (emb @ w_ih1[:, :H].T + (b_ih1 + b_hh1)[None, :])[:, permg] * rs1
    ids = np.concatenate([np.zeros((1, B), np.int64), text[1:nsteps]], axis=0)
    gih_all = tab1[ids]                                        # [ns, B, 4H]

    whh1_p = 0.5 * w_hh1[permg] * rs1[:, None]                 # [2048, 512]
    whh1T = whh1_p.T.reshape(4, 128, G1).transpose(1, 0, 2)    # [128,4,2048]
    wih1cT = (w_ih1[permg][:, H:H + V] * rs1[:, None]).T.copy()  # [128,2048]
    wih2_p = 0.5 * w_ih2[permg2] * rs2[:, None]                # [512, 512]
    wih2T = wih2_p.T.reshape(4, 128, G2).transpose(1, 0, 2)    # [128,4,512]
    whh2T = (0.5 * w_hh2[permg2] * rs2[:, None]).T.copy()      # [128,512]
    woutT = np.stack([0.5 * w_out[:, 0:K].T, w_out[:, K:K + V].T], axis=1)

    m01 = (np.arange(S)[None, :] < lens[:, None]).astype(f32)  # [B, S]

    dmT = np.zeros((128, BC, BC), f32)                         # [p, m, b]
    for b in range(BC):
        dmT[:, b, b] = 1.0

    consts = dict(
        whh1T=whh1T.astype(BF16N), wih1cT=wih1cT.astype(BF16N),
        wih2T=wih2T.astype(BF16N), whh2T=whh2T.astype(BF16N),
        woutT=woutT.astype(BF16N),
        dmT=dmT.reshape(128, BC * BC).astype(BF16N),
        i16b=np.eye(16, dtype=BF16N),
    )

    nch_all = np.clip((lens + 127) // 128, 1, 4).astype(np.int64)

    # Sort batch by length so local slot b holds similar lengths on every
    # core (ranks b*NC .. b*NC+NC-1) -> per-slot chunk counts stay small.
    order = np.argsort(lens, kind="stable")                    # [B]
    assign = order.reshape(BC, NC)                             # [slot, core]

    in_maps = []
    for i in range(NC):
        bs = assign[:, i]                                      # [BC] global b
        kkc = np.zeros((128, BC, SP), f32)
        kkc[:, :, :S] = 0.5 * key[:, bs, :].transpose(2, 1, 0)
        vvc = np.zeros((128, 4, BC, V + 2), f32)
        vals_m = values[:, bs, :] * m01.T[:, bs, None]         # [S, BC, V]
        vpad = np.zeros((4 * 128, BC, V + 2), f32)
        vpad[:S, :, :V] = vals_m
        vpad[:S, :, V] = m01.T[:, bs]
        vvc[:, :, :, :] = vpad.reshape(4, 128, BC, V + 2).transpose(1, 0, 2, 3)
        in_maps.append(dict(
            consts,
            kk=kkc.astype(BF16N),
            vv=vvc.astype(BF16N),
            gih=gih_all[:, bs, :].transpose(1, 0, 2).copy().astype(BF16N),
        ))
    nch = tuple(int(nch_all[assign[b, :]].max()) for b in range(BC))
    return in_maps, b_out, nch, assign


def kernel(**inputs):
    from concourse.bass_utils import run_bass_kernel_spmd

    nsteps = inputs.pop("_nsteps", TS)
    in_maps, b_out, nch, assign = _host_prep(nsteps=nsteps, **inputs)
    key_ = (nsteps, nch)
    if key_ not in _BUILT:
        _BUILT[key_] = _build(nsteps, nch)
        _BUILT[nsteps] = _BUILT[key_]   # test.py indexes by nsteps
    nc = _BUILT[key_]

    res = run_bass_kernel_spmd(nc, in_maps, list(range(NC)))
    out = np.empty((B, nsteps, VOCAB), np.float32)
    for i in range(NC):
        out[assign[:, i]] = res.results[i]["preds"].astype(np.float32)
    out += b_out[None, None, :]
    return out
